# revision 28
# baseline (speedup 1.0000x reference)
"""3-layer GCN (PyG GCNConv + BatchNorm + ReLU) on 8 Trainium2 NeuronCores.

Strategy (edge-parallel via dst-range sharding), v2:
  - Nodes are sharded 8 ways (core = v // 12500).  Within each core, dsts are
    assigned to 98 windows of 128 psum columns by STRATIFIED degree order
    (sort by in-degree, deal round-robin), so per-(window, chunk) edge counts
    are nearly identical across windows AND cores; the shared SPMD tile
    schedule then pads ~10% instead of ~27%.
  - The per-layer fp16 table (hw' = (h @ W) * dinv, node-major) is built in 4
    window-quarters, each AllGather'd into its own shared buffer as soon as
    its phase-A matmuls finish, so chunk 0's gathers overlap the remaining
    quarters' broadcast.
  - Per layer, per core, phase B: batched SWDGE dma_gather of hw'[src] per
    edge (int16 idxs relative to the edge's table quarter), then per-tile
    TensorE one-hot matmul accumulating psum[f, dstcol] over the cell's
    tiles; one DVE add per (chunk, window) into aggT.
  - Phase C: BN stats via bn_stats/bn_aggr, tiny AllReduce, fused
    scale+shift+ReLU on ScalarE producing the next layer's fp16 hT.

Self-loops are plain edges: value dinv[v] in the one-hot against table row
hw[v]*dinv[v] gives exactly the dinv^2[v]*hw[v] self term of the reference.
The bias b is absorbed by BatchNorm's mean subtraction and skipped.
"""

import os
import sys
import time

for _p in ("/opt/trn_rl_repo",):
    if _p not in sys.path:
        sys.path.insert(0, _p)

import numpy as np
from contextlib import ExitStack

import concourse.bacc as bacc
import concourse.bass as bass
import concourse.tile as tile
from concourse import mybir
from concourse.bass_utils import run_bass_kernel_spmd

# problem constants (hardcoded per spec nn_GCNLayers_15607911154176)
N = 100000
D = 128
NCORES = 8
SHARD = 12500           # nodes per core
NWIN = 98               # windows per shard
WIN = 128               # dst window width
SPAD = NWIN * WIN       # 12544 slots per shard
NQRT = 4                # table quarters == gather chunks
QWIN = [25, 25, 24, 24]         # windows per quarter
QW0 = [0, 25, 50, 74]           # first window of each quarter
QSLOT = [w * WIN for w in QWIN]  # slots per core per quarter
CHUNK_ROWS = [NCORES * s for s in QSLOT]   # table rows per chunk
CHUNK_BASE = [0]
for _s in CHUNK_ROWS[:-1]:
    CHUNK_BASE.append(CHUNK_BASE[-1] + _s)
# dma_gather tolerates at most 1024 idxs per instruction on HW.
GB = int(os.environ.get("KERNEL_GB", "8"))
NQ = int(os.environ.get("KERNEL_NQ", "4"))
GBUFS = int(os.environ.get("KERNEL_GBUFS", "18"))
OHBUFS = int(os.environ.get("KERNEL_OHBUFS", "8"))
BALANCE = os.environ.get("KERNEL_BALANCE", "1") == "1"
SINGLE_PACKET = os.environ.get("KERNEL_SP", "1") == "1"
PHASES = os.environ.get("KERNEL_PHASES", "ABC")
NLAYERS = int(os.environ.get("KERNEL_NLAYERS", "3"))
BN_EPS = 1e-5
F16 = mybir.dt.float16
F32 = mybir.dt.float32
I16 = mybir.dt.int16


# ---------------------------------------------------------------- schedule

def make_schedule(counts_per_core: np.ndarray):
    """counts_per_core: [NCORES, NQRT*NWIN] edge counts per (chunk, window).
    Returns (K, tiles, batches): K tiles per cell (max over cores), tiles in
    (chunk-major, window) order, gather batches of <=GB tiles per chunk."""
    K = np.ceil(counts_per_core.max(axis=0) / 128).astype(np.int64)
    K = np.maximum(K, 1)
    tiles = []
    batches = []
    for c in range(NQRT):
        chunk_t0 = len(tiles)
        for w in range(NWIN):
            k = int(K[c * NWIN + w])
            for j in range(k):
                tiles.append((c, w, j, k))
        t = chunk_t0
        while t < len(tiles):
            nb = min(GB, len(tiles) - t)
            batches.append((c, t, nb))
            t += nb
    return K, tiles, batches


# ---------------------------------------------------------------- device code

def build_program(tiles, batches, ntiles):
    nc = bacc.Bacc("TRN2", target_bir_lowering=False, debug=False,
                   num_devices=NCORES, num_swdge_queues=NQ)

    xT_p = nc.declare_dram_parameter("xT", [128, SPAD], F16, isOutput=False)
    idx_p = nc.declare_dram_parameter("idx", [128, ntiles * 8], I16, isOutput=False)
    oh_p = nc.declare_dram_parameter("oh", [128, ntiles, WIN], F16, isOutput=False)
    dinv_p = nc.declare_dram_parameter("dinv_nm", [128, NWIN], F32, isOutput=False)
    w_ps = [nc.declare_dram_parameter(f"w{l}", [128, 128], F16, isOutput=False)
            for l in range(3)]
    gb_ps = [nc.declare_dram_parameter(f"gb{l}", [128, 2], F32, isOutput=False)
             for l in range(3)]
    out_p = nc.declare_dram_parameter("hT_out", [128, SPAD], F16, isOutput=True)

    shard_d = [[nc.dram_tensor(f"shard{l}_{q}", [QSLOT[q], 128], F16)
                for q in range(NQRT)] for l in range(3)]
    table_d = [[nc.dram_tensor(f"table{l}_{q}", [CHUNK_ROWS[q], 128], F16,
                               addr_space="Shared")
                for q in range(NQRT)] for l in range(3)]
    stats_in_d = [nc.dram_tensor(f"stats_in{l}", [128, 2], F32) for l in range(3)]
    stats_rd_d = [nc.dram_tensor(f"stats_rd{l}", [128, 2], F32, addr_space="Shared")
                  for l in range(3)]

    rg = [list(range(NCORES))]

    with tile.TileContext(nc) as tc, ExitStack() as ctx:
        persist = ctx.enter_context(tc.tile_pool(name="persist", bufs=1))
        gpool = ctx.enter_context(tc.tile_pool(name="gpool", bufs=GBUFS))
        ohpool = ctx.enter_context(tc.tile_pool(name="ohpool", bufs=OHBUFS))
        stpool = ctx.enter_context(tc.tile_pool(name="stpool", bufs=4))
        scal = ctx.enter_context(tc.tile_pool(name="scal", bufs=4))
        psum_w = ctx.enter_context(tc.tile_pool(name="psum_w", bufs=6, space="PSUM"))
        psum_g = ctx.enter_context(tc.tile_pool(name="psum_g", bufs=2, space="PSUM"))

        hT = persist.tile([128, SPAD], F16)
        aggT = persist.tile([128, SPAD], F32)
        idx_sb = persist.tile([128, ntiles * 8], I16)
        dinv_sb = persist.tile([128, NWIN], F32)
        w_sb = [persist.tile([128, 128], F16, name=f"wsb{l}", tag=f"w{l}")
                for l in range(3)]
        gb_sb = [persist.tile([128, 2], F32, name=f"gbsb{l}", tag=f"gb{l}")
                 for l in range(3)]
        eps_sb = persist.tile([128, 1], F32)
        bn6 = persist.tile([128, NWIN, 6], F32)

        nc.sync.dma_start(out=hT[:], in_=xT_p[:])
        nc.sync.dma_start(out=idx_sb[:], in_=idx_p[:])
        nc.sync.dma_start(out=dinv_sb[:], in_=dinv_p[:])
        for l in range(3):
            nc.sync.dma_start(out=w_sb[l][:], in_=w_ps[l][:])
            nc.sync.dma_start(out=gb_sb[l][:], in_=gb_ps[l][:])
        nc.vector.memset(eps_sb[:], BN_EPS)

        for l in range(NLAYERS):
            # ---- phase A: per quarter, shard = (h @ W) * dinv, then AllGather
            for q in range(NQRT):
                shard_v = shard_d[l][q].ap().rearrange("(b p) f -> p b f", p=128)
                for j in range(QWIN[q]):
                    b = QW0[q] + j
                    ps = psum_g.tile([128, 128], F32)
                    nc.tensor.matmul(
                        out=ps[:], lhsT=hT[:, b * 128:(b + 1) * 128],
                        rhs=w_sb[l][:], start=True, stop=True)
                    st = stpool.tile([128, 128], F16, tag="stage")
                    nc.vector.tensor_scalar_mul(st[:], ps[:], dinv_sb[:, b:b + 1])
                    nc.sync.dma_start(out=shard_v[:, j, :], in_=st[:])
                nc.gpsimd.collective_compute(
                    "AllGather", mybir.AluOpType.bypass, replica_groups=rg,
                    ins=[shard_d[l][q][:]], outs=[table_d[l][q][:]])

            # ---- phase B: gather + one-hot aggregate
            if "B" not in PHASES:
                continue
            nc.vector.memset(aggT[:], 0.0)
            bmode = os.environ.get("KERNEL_B_MODE", "full")
            ti = 0  # global tile cursor (tiles are in batch order)
            for bi, (c, t0, nb) in enumerate(batches):
                g = gpool.tile([128, GB, 128], F16, tag="g")
                if bmode != "mm":
                    src_tab = table_d[0][0] if bmode == "gonly0" else table_d[l][c]
                    nc.gpsimd.dma_gather(
                        g[:, :nb, :],
                        src_tab[:CHUNK_ROWS[c], :] if bmode == "gonly0" else src_tab[:],
                        idx_sb[:, t0 * 8:(t0 + nb) * 8],
                        nb * 128, nb * 128, 128,
                        queue_num=bi % NQ, single_packet=SINGLE_PACKET,
                    )
                if bmode.startswith("gonly"):
                    ti += nb
                    continue
                oh = ohpool.tile([128, GB, WIN], F16, tag="oh")
                nc.sync.dma_start(out=oh[:, :nb, :], in_=oh_p[:, t0:t0 + nb, :])
                if bmode == "gather":
                    ti += nb
                    continue
                for t in range(nb):
                    (tc_, tw, tk, tkmax) = tiles[ti]
                    assert tc_ == c
                    if tk == 0:
                        pw = psum_w.tile([128, WIN], F32, tag="pw")
                    nc.tensor.matmul(
                        out=pw[:], lhsT=g[:, t, :], rhs=oh[:, t, :],
                        start=(tk == 0), stop=(tk == tkmax - 1))
                    if tk == tkmax - 1:
                        nc.vector.tensor_add(
                            aggT[:, tw * WIN:(tw + 1) * WIN],
                            aggT[:, tw * WIN:(tw + 1) * WIN],
                            pw[:])
                        if c == NQRT - 1 and "C" in PHASES:
                            # window tw is final: fold its BN stats in now so
                            # phase C's serial tail shrinks
                            nc.vector.bn_stats(
                                out=bn6[:, tw, :],
                                in_=aggT[:, tw * WIN:(tw + 1) * WIN])
                    ti += 1
            if bmode == "full":
                assert ti == len(tiles)

            # ---- phase C: BN stats + AllReduce + finalize
            if "C" not in PHASES:
                continue
            mv = scal.tile([128, 2], F32, tag="mv")
            nc.vector.bn_aggr(out=mv[:], in_=bn6[:])
            # S1 = mean * SPAD ; S2 = (var + mean^2) * SPAD
            st2 = scal.tile([128, 2], F32, tag="st2")
            m2 = scal.tile([128, 1], F32, tag="m2")
            nc.vector.tensor_mul(m2[:], mv[:, 0:1], mv[:, 0:1])
            nc.vector.tensor_scalar_mul(st2[:, 0:1], mv[:, 0:1], float(SPAD))
            nc.vector.tensor_add(m2[:], mv[:, 1:2], m2[:])
            nc.vector.tensor_scalar_mul(st2[:, 1:2], m2[:], float(SPAD))
            nc.sync.dma_start(out=stats_in_d[l][:], in_=st2[:])
            nc.gpsimd.collective_compute(
                "AllReduce", mybir.AluOpType.add, replica_groups=rg,
                ins=[stats_in_d[l][:]], outs=[stats_rd_d[l][:]])
            sr = scal.tile([128, 2], F32, tag="sr")
            nc.sync.dma_start(out=sr[:], in_=stats_rd_d[l][:])

            mu = scal.tile([128, 1], F32, tag="mu")
            var = scal.tile([128, 1], F32, tag="var")
            nc.vector.tensor_scalar_mul(mu[:], sr[:, 0:1], 1.0 / N)
            nc.vector.tensor_scalar_mul(var[:], sr[:, 1:2], 1.0 / N)
            t1 = scal.tile([128, 1], F32, tag="t1")
            nc.vector.tensor_mul(t1[:], mu[:], mu[:])
            nc.vector.tensor_sub(var[:], var[:], t1[:])
            sd = scal.tile([128, 1], F32, tag="sd")
            nc.scalar.activation(out=sd[:], in_=var[:],
                                 func=mybir.ActivationFunctionType.Sqrt,
                                 bias=eps_sb[:], scale=1.0)
            r = scal.tile([128, 1], F32, tag="r")
            nc.vector.reciprocal(out=r[:], in_=sd[:])
            scale = scal.tile([128, 1], F32, tag="scale")
            shift = scal.tile([128, 1], F32, tag="shift")
            nc.vector.tensor_mul(scale[:], gb_sb[l][:, 0:1], r[:])
            nc.vector.tensor_mul(t1[:], mu[:], scale[:])
            nc.vector.tensor_sub(shift[:], gb_sb[l][:, 1:2], t1[:])
            # h_next = relu(agg * scale + shift)
            nc.scalar.activation(out=hT[:], in_=aggT[:],
                                 func=mybir.ActivationFunctionType.Relu,
                                 bias=shift[:], scale=scale[:])

        nc.sync.dma_start(out=out_p[:], in_=hT[:])

    nc.compile()
    return nc


# ---------------------------------------------------------------- host side

def assign_windows(deg_in):
    """Per-core stratified dst->(window, col) assignment.
    Returns wcol[N] = w * 128 + col per node (slot within its core's shard)."""
    wcol = np.empty(N, dtype=np.int64)
    for i in range(NCORES):
        d = deg_in[i * SHARD:(i + 1) * SHARD]
        if BALANCE:
            order = np.argsort(-d, kind="stable")
        else:
            order = np.arange(SHARD)
        r = np.empty(SHARD, dtype=np.int64)
        r[order] = np.arange(SHARD)
        w = r % NWIN
        col = r // NWIN
        wcol[i * SHARD:(i + 1) * SHARD] = w * 128 + col
    return wcol


def repair_windows(wcol, deg_vec, limit=640, sweeps=3):
    """Swap dsts between same-quarter windows (per core) until every
    (window, chunk) degree sum is <= limit.  Same-quarter swaps keep each
    node's table quarter (hence its out-edges' chunks) stable."""
    w_all = wcol // 128
    for i in range(NCORES):
        nodes = np.arange(i * SHARD, (i + 1) * SHARD)
        w = w_all[nodes].copy()
        cnt = np.zeros((NWIN, NQRT), np.int64)
        np.add.at(cnt, w, deg_vec[nodes])
        for _ in range(sweeps):
            moved = False
            for q in range(NQRT):
                wlo, whi = QW0[q], QW0[q] + QWIN[q]
                for c in range(NQRT):
                    for _ in range(200):
                        sub = cnt[wlo:whi, c]
                        wo = wlo + int(np.argmax(sub))
                        if cnt[wo, c] <= limit:
                            break
                        wt = wlo + int(np.argmin(sub))
                        m_o = np.where(w == wo)[0]
                        m_t = np.where(w == wt)[0]
                        v = m_o[np.argmax(deg_vec[nodes[m_o], c])]
                        u = m_t[np.argmin(deg_vec[nodes[m_t], c])]
                        cnt[wo] += deg_vec[nodes[u]] - deg_vec[nodes[v]]
                        cnt[wt] += deg_vec[nodes[v]] - deg_vec[nodes[u]]
                        wcol[nodes[v]], wcol[nodes[u]] = wcol[nodes[u]], wcol[nodes[v]]
                        w[v], w[u] = w[u], w[v]
                        moved = True
            if not moved:
                break
    return wcol


def preprocess(edge_index, dinv):
    """Build per-core input maps (idx/oh) + the shared tile schedule."""
    src = np.asarray(edge_index[0], dtype=np.int64)
    dst = np.asarray(edge_index[1], dtype=np.int64)
    deg_in = np.bincount(dst, minlength=N)

    wcol = assign_windows(deg_in)     # slot within shard
    core_of = np.arange(N) // SHARD
    # quarter of a window
    q_of_w = np.zeros(NWIN, dtype=np.int64)
    for q in range(NQRT):
        q_of_w[QW0[q]:QW0[q] + QWIN[q]] = q
    if BALANCE:
        # per-(dst, src-quarter) degree incl self-loop; src quarters are
        # fixed under same-quarter swaps
        q_src = q_of_w[wcol[src] // 128]
        deg_vec = np.zeros((N, NQRT), np.int64)
        np.add.at(deg_vec, (dst, q_src), 1)
        deg_vec[np.arange(N), q_of_w[wcol // 128]] += 1
        wcol = repair_windows(wcol, deg_vec)
    w_of = wcol // 128
    q_of = q_of_w[w_of]
    # table position: chunk_base + core*qslots + (w - qw0)*128 + col
    qslot_arr = np.array(QSLOT)[q_of]
    qw0_arr = np.array(QW0)[q_of]
    cb_arr = np.array(CHUNK_BASE)[q_of]
    pos = cb_arr + core_of * qslot_arr + (w_of - qw0_arr) * 128 + (wcol % 128)

    # append self-loops
    loops = np.arange(N, dtype=np.int64)
    src_a = np.concatenate([src, loops])
    dst_a = np.concatenate([dst, loops])

    core = core_of[dst_a]
    c = q_of[src_a]                         # chunk of the edge = src quarter
    srcl = (pos[src_a] - np.array(CHUNK_BASE)[c]).astype(np.int64)
    w = w_of[dst_a]
    col = wcol[dst_a] % 128
    cell = c * NWIN + w

    counts = np.zeros((NCORES, NQRT * NWIN), dtype=np.int64)
    per_core = []
    for i in range(NCORES):
        m = core == i
        cell_i = cell[m]
        counts[i] = np.bincount(cell_i, minlength=NQRT * NWIN)
        per_core.append((cell_i, srcl[m], col[m], dst_a[m]))

    K, tiles, batches = make_schedule(counts)
    ntiles = len(tiles)
    cell_tile_base = np.zeros(NQRT * NWIN, dtype=np.int64)
    acc = 0
    for cc in range(NQRT):
        for ww in range(NWIN):
            cell_tile_base[cc * NWIN + ww] = acc
            acc += int(K[cc * NWIN + ww])
    assert acc == ntiles

    in_maps = []
    for i in range(NCORES):
        cell_i, srcl_i, col_i, dsta_i = per_core[i]
        order = np.argsort(cell_i, kind="stable")
        cell_s = cell_i[order]
        srcl_s = srcl_i[order]
        col_s = col_i[order]
        dst_s = dsta_i[order]
        cnts = np.bincount(cell_s, minlength=NQRT * NWIN)
        starts = np.zeros(NQRT * NWIN, dtype=np.int64)
        starts[1:] = np.cumsum(cnts)[:-1]
        within = np.arange(cell_s.shape[0]) - starts[cell_s]
        slot = cell_tile_base[cell_s] * 128 + within

        # pad slots must point somewhere harmless (their one-hot rows are
        # zero); spread them uniformly so no HBM bank becomes a hotspot
        rngp = np.random.default_rng(12345 + i)
        lim = np.array(CHUNK_ROWS)[np.array([t[0] for t in tiles])]  # [ntiles]
        idx_arr = (rngp.random(ntiles * 128) * np.repeat(lim, 128)).astype(np.int16)
        idx_arr[slot] = srcl_s
        oh_arr = np.zeros((128, ntiles, WIN), dtype=np.float16)
        oh_arr[slot % 128, slot // 128, col_s] = dinv[dst_s].astype(np.float16)

        wrapped = idx_arr.reshape(-1, 16).T  # [16, ntiles*8]
        idx_packed = np.tile(wrapped, (8, 1)).copy()  # [128, ntiles*8]
        in_maps.append({"idx": idx_packed, "oh": oh_arr})
    return in_maps, tiles, batches, ntiles, wcol


_CACHE = {}
LAST_EXEC_NS = None


def kernel(**inputs) -> np.ndarray:
    x = np.asarray(inputs["x"], dtype=np.float32)
    edge_index = np.asarray(inputs["edge_index"], dtype=np.int64)
    assert x.shape == (N, D) and edge_index.shape[1:] == (1600000,)

    deg = np.bincount(edge_index[1], minlength=N).astype(np.float64) + 1.0
    dinv = (1.0 / np.sqrt(deg)).astype(np.float32)

    in_maps, tiles, batches, ntiles, wcol = preprocess(edge_index, dinv)

    ck = ("prog2", ntiles, tuple(t[0] * 1000 + t[1] for t in tiles[::97]))
    if ck in _CACHE:
        nc = _CACHE[ck]
    else:
        t0 = time.time()
        nc = build_program(tiles, batches, ntiles)
        print(f"[kernel] build+compile {time.time()-t0:.1f}s "
              f"ntiles={ntiles} nbatches={len(batches)}", flush=True)
        _CACHE.clear()
        _CACHE[ck] = nc

    # per-core node-major arrays in (window, col) slot order
    slot_of = wcol  # slot within shard
    for i in range(NCORES):
        im = in_maps[i]
        nodes = np.arange(i * SHARD, (i + 1) * SHARD)
        s = slot_of[nodes]
        # hT layout: column index = slot within shard, partition = feature
        xcols = np.zeros((SPAD, 128), dtype=np.float16)
        xcols[s] = x[nodes].astype(np.float16)
        im["xT"] = xcols.T.copy()
        dv = np.zeros(SPAD, dtype=np.float32)
        dv[s] = dinv[nodes]
        im["dinv_nm"] = dv.reshape(NWIN, 128).T.copy()
        for l in range(3):
            im[f"w{l}"] = np.asarray(inputs[f"W{l}"], dtype=np.float16)
            gamma = np.asarray(inputs[f"gamma{l}"], dtype=np.float32)
            beta = np.asarray(inputs[f"beta{l}"], dtype=np.float32)
            im[f"gb{l}"] = np.stack([gamma, beta], axis=1).copy()

    if os.environ.get("KERNEL_RAND_IDX", "0") == "1":
        rng = np.random.default_rng(7)
        for im in in_maps:
            ri = rng.integers(0, 24576, size=(ntiles * 128,)).astype(np.int16)
            w2 = ri.reshape(-1, 16).T
            im["idx"] = np.tile(w2, (8, 1)).copy()

    t0 = time.time()
    res = run_bass_kernel_spmd(nc, in_maps, list(range(NCORES)))
    print(f"[kernel] run {time.time()-t0:.1f}s", flush=True)
    global LAST_EXEC_NS
    LAST_EXEC_NS = res.exec_time_ns

    out = np.empty((N, D), dtype=np.float32)
    for i in range(NCORES):
        hT = res.results[i]["hT_out"].astype(np.float32)  # [128, SPAD]
        nodes = np.arange(i * SHARD, (i + 1) * SHARD)
        s = slot_of[nodes]
        out[nodes] = hT[:, s].T
    return out


if __name__ == "__main__":
    rng = np.random.default_rng(0)
    ins = {
        "x": rng.standard_normal((N, D)).astype(np.float32),
        "edge_index": rng.integers(0, N, size=(2, 1600000)),
    }
    for l in range(3):
        ins[f"W{l}"] = ((rng.random((128, 128), dtype=np.float32) - 0.5)
                        / np.sqrt(128)).astype(np.float32)
        ins[f"b{l}"] = np.zeros(128, np.float32)
        ins[f"gamma{l}"] = np.ones(128, np.float32)
        ins[f"beta{l}"] = np.zeros(128, np.float32)
    out = kernel(**ins)
    print("out", out.shape, out.dtype, float(np.abs(out).max()))
